# revision 20
# baseline (speedup 1.0000x reference)
"""Distributed Bass kernel for sliding-window GQA attention on 8 TRN2 NeuronCores.

Problem: B=2, S=2048, DIM=2048, H=16, KVH=4, HD=128, WINDOW=1024 (causal
sliding window), nonstandard RoPE producing 1.5*HD score features.

Sharding (tensor-parallel on the kv-head axis, data-parallel on batch —
no collectives): core c owns (batch, kv-group) = (c//4, c%4): its 4 q-heads
and 1 kv head over the full 2048-row sequence. wq/wk/wv are column-sharded
by kv group, wo row-sharded. Each core emits a PARTIAL output projection
(its 4 heads x its wo rows); the host sums the 4 partials per batch while
unsharding — replacing the all-reduce.

Structure: scores are computed TRANSPOSED (S^T[k, q], k on partitions) with
all 4 heads packed into one N=512 matmul pair per k-block — q1/k1 are
feature-major already so this is free. The imag-half (64-dim) contraction
is duplicated across both partition halves (wq imag columns pre-halved on
the host) so both score passes run K=128 and keep fast-weight-load. The
softmax row-sums come from a ones-column appended to V (PV out [q, 129]
carries the denominator in col 128), normalization happens during the
PSUM->SBUF attn copy (per-partition scalar mul), and a single 128x128
transpose matmul per (head, q-block) feeds the O-projection.

Pipelining: attention for q-blocks 4cq..4cq+3 is interleaved right after
column-chunk cq's projections; within a q-block the PV groups trail the
score matmuls by 2 k-blocks (so the exp on the Scalar engine never stalls
the in-order PE queue), and the normalize/transpose/O-proj of q-block qc
is emitted inside q-block qc+1's loop to keep the PE fed at qc boundaries.
"""
import numpy as np
import ml_dtypes

import concourse.tile as tile
from concourse import bacc, mybir
from concourse.bass_utils import run_bass_kernel_spmd
from contextlib import ExitStack

F32 = mybir.dt.float32
BF16 = mybir.dt.bfloat16
EXP = mybir.ActivationFunctionType.Exp

B, S, DIM = 2, 2048, 2048
H, KVH, HD = 16, 4, 128
HPC = H // KVH  # heads per core (4)
WINDOW = 1024
SCALE = HD ** -0.5
NDC = DIM // 128  # 16 dim chunks
NQC = S // 128    # 16 q blocks

_cache = {}


def _kblocks(qc):
    return list(range(max(0, qc - 8), qc + 1))


def _build():
    nc = bacc.Bacc("TRN2", target_bir_lowering=False, debug=False, num_devices=8)

    xt_d = nc.dram_tensor("xt", [128, 4 * NDC * 512], BF16, kind="ExternalInput")
    wq_d = nc.dram_tensor("wq", [128, 2 * NDC * 256], BF16, kind="ExternalInput")
    wkv_d = nc.dram_tensor("wkv", [128, NDC * 256], BF16, kind="ExternalInput")
    wo_d = nc.dram_tensor("wo", [128, HPC * 2048], BF16, kind="ExternalInput")
    fm_d = nc.dram_tensor("fm", [64, S], F32, kind="ExternalInput")
    fp_d = nc.dram_tensor("fp", [64, S], F32, kind="ExternalInput")
    t0_d = nc.dram_tensor("t0", [128, 512], F32, kind="ExternalInput")
    t8_d = nc.dram_tensor("t8", [128, 512], F32, kind="ExternalInput")
    id_d = nc.dram_tensor("ident", [128, 128], BF16, kind="ExternalInput")
    on_d = nc.dram_tensor("vones", [128, NQC], BF16, kind="ExternalInput")
    out_d = nc.dram_tensor("out", [S, DIM], BF16, kind="ExternalOutput")

    with tile.TileContext(nc) as tc, ExitStack() as ctx:
        xp = ctx.enter_context(tc.tile_pool(name="xp", bufs=3))
        wp = ctx.enter_context(tc.tile_pool(name="wp", bufs=1))
        cp = ctx.enter_context(tc.tile_pool(name="cp", bufs=1))
        qp = ctx.enter_context(tc.tile_pool(name="qp", bufs=1))
        kp = ctx.enter_context(tc.tile_pool(name="kp", bufs=1))
        vp = ctx.enter_context(tc.tile_pool(name="vp", bufs=1))
        pp = ctx.enter_context(tc.tile_pool(name="pp", bufs=6))
        ap_ = ctx.enter_context(tc.tile_pool(name="ap", bufs=8))
        atp = ctx.enter_context(tc.tile_pool(name="atp", bufs=2))
        rp = ctx.enter_context(tc.tile_pool(name="rp", bufs=8))
        op_ = ctx.enter_context(tc.tile_pool(name="op", bufs=2))
        # PSUM: 8 banks = ps(2: proj + O-proj) + sps(3: scores + attn
        # transposes) + pvs(3: PV accumulators, 2 heads per bank)
        ps = ctx.enter_context(tc.tile_pool(name="ps", bufs=2, space="PSUM"))
        sps = ctx.enter_context(tc.tile_pool(name="sps", bufs=3, space="PSUM"))
        pvs = ctx.enter_context(tc.tile_pool(name="pvs", bufs=3, space="PSUM"))

        # ---- persistent SBUF tensors ----
        q1 = qp.tile([128, NQC, 512], BF16, tag="q1")  # [feat, qblock, h*128+q]
        q2 = qp.tile([128, NQC, 512], BF16, tag="q2")  # imag, duplicated halves
        k1 = kp.tile([128, S], BF16, tag="k1")
        k2 = kp.tile([128, S], BF16, tag="k2")  # imag, duplicated halves
        v_sb = vp.tile([128, NQC, 132], BF16, tag="v")  # col 128 = ones

        # weights for phase 1 first (prologue-critical DMA order)
        wkv_t = wp.tile([128, NDC, 256], BF16, tag="wkv")  # [wk 128 | wv 128]
        for i in range(2):
            nc.sync.dma_start(
                wkv_t[:, i * 8 : (i + 1) * 8, :],
                wkv_d[:, i * 8 * 256 : (i + 1) * 8 * 256],
            )
        # prewarm the exp table-set load (~2.7us) off the first-landing DMA —
        # a later input here would head-of-line-block the Scalar FIFO
        warm = rp.tile([128, 1], F32, tag="rc", name="warm")
        nc.scalar.activation(warm[:], wkv_t[:, 0, 0:1], EXP)
        # tiny constants next: anything queued late here head-of-line-blocks
        # the consumer engine FIFOs behind ~35us of bulk input DMA
        von = cp.tile([128, NQC], BF16, tag="vones")
        nc.sync.dma_start(von[:], on_d[:, :])
        ident = cp.tile([128, 128], BF16, tag="ident")
        nc.sync.dma_start(ident[:], id_d[:, :])
        t0 = cp.tile([128, 512], F32, tag="t0")
        nc.sync.dma_start(t0[:], t0_d[:, :])
        t8 = cp.tile([128, 512], F32, tag="t8")
        nc.sync.dma_start(t8[:], t8_d[:, :])

        def load_x(cq):
            x_q = xp.tile([128, NDC, 512], BF16, tag="x", name=f"x{cq}")
            ndg = 8 if cq == 0 else 2
            w_dg = NDC // ndg
            for dg in range(ndg):
                nc.sync.dma_start(
                    x_q[:, dg * w_dg : (dg + 1) * w_dg, :],
                    xt_d[
                        :,
                        cq * NDC * 512 + dg * w_dg * 512 : cq * NDC * 512
                        + (dg + 1) * w_dg * 512,
                    ],
                )
            return x_q

        x_tiles = {0: load_x(0)}
        wq_t = fm = fp = wo_t = None
        pending = None  # (pv tiles, qc) awaiting normalize/transpose/O-proj

        def emit_finish(st):
            pv, qc = st
            qb = qc * 128
            at_sb = atp.tile([128, HPC, 128], BF16, tag="at")
            for h in range(HPC):
                c0 = (h % 2) * 132
                rc = rp.tile([128, 1], F32, tag="rc")
                nc.vector.reciprocal(rc[:], pv[h // 2][:, c0 + 128 : c0 + 129])
                a_sb = ap_.tile([128, 128], BF16, tag="a")
                nc.vector.tensor_scalar_mul(
                    a_sb[:], pv[h // 2][:, c0 : c0 + 128], rc[:, 0:1]
                )
                tps = sps.tile([128, 512], F32, tag="s", name=f"tp{h}")
                nc.tensor.matmul(
                    tps[:, 0:128], a_sb[:], ident[:], start=True, stop=True
                )
                # alternate engines so the 4 copies gating the O-proj drain
                # two-wide instead of queuing on one engine
                if h % 2 == 0:
                    nc.scalar.copy(at_sb[:, h, :], tps[:, 0:128])
                else:
                    nc.vector.tensor_copy(at_sb[:, h, :], tps[:, 0:128])
            o_sb = op_.tile([128, 2048], BF16, tag="o")
            for dn in range(4):
                ops = ps.tile([128, 512], F32, tag="ps")
                for f in range(HPC):
                    nc.tensor.matmul(
                        ops[:],
                        at_sb[:, f, :],
                        wo_t[:, f, dn * 512 : (dn + 1) * 512],
                        start=(f == 0),
                        stop=(f == HPC - 1),
                    )
                nc.vector.tensor_copy(o_sb[:, dn * 512 : (dn + 1) * 512], ops[:])
            # one fused DMA per q-block: each dma_start costs ~600ns of Sync
            # sequencing, which serializes against every other DMA issue
            nc.sync.dma_start(out_d[qb : qb + 128, :], o_sb[:])

        for cq in range(4):
            # ---- projections for column-quarter cq ----
            x_q = x_tiles.pop(cq)
            if cq == 0:
                # wq + constants ride after the first x chunk
                wq_t = [
                    wp.tile([128, NDC, 256], BF16, tag=f"wq{i}", name=f"wq{i}")
                    for i in range(2)
                ]
                for i in range(2):
                    nc.sync.dma_start(
                        wq_t[i][:], wq_d[:, i * NDC * 256 : (i + 1) * NDC * 256]
                    )
                fm = cp.tile([64, S], F32, tag="fm")
                nc.sync.dma_start(fm[:], fm_d[:, :])
                fp = cp.tile([64, S], F32, tag="fp")
                nc.sync.dma_start(fp[:], fp_d[:, :])
                nc.vector.tensor_copy(v_sb[:, :, 128:129], von[:].unsqueeze(2))
                wo_t = wp.tile([128, HPC, 2048], BF16, tag="wo")
                for i in range(2):
                    nc.sync.dma_start(
                        wo_t[:, i * 2 : (i + 1) * 2, :],
                        wo_d[:, i * 4096 : (i + 1) * 4096],
                    )
            cs = slice(cq * 512, (cq + 1) * 512)
            fmc, fpc = fm[:, cs], fp[:, cs]

            # K projection + rope
            kps = ps.tile([128, 512], F32, tag="ps")
            for dc in range(NDC):
                nc.tensor.matmul(
                    kps[:],
                    wkv_t[:, dc, 0:128],
                    x_q[:, dc, :],
                    start=(dc == 0),
                    stop=(dc == NDC - 1),
                )
            nc.vector.tensor_mul(k1[0:64, cs], kps[0:64, :], fmc)
            nc.vector.tensor_mul(k1[64:128, cs], kps[0:64, :], fpc)
            nc.scalar.copy(k2[0:64, cs], kps[64:128, :])
            nc.scalar.copy(k2[64:128, cs], kps[64:128, :])

            # Q projections + rope, packed layout [feat, qblock, h*128+q]
            for h in range(HPC):
                qps = ps.tile([128, 512], F32, tag="ps")
                for dc in range(NDC):
                    nc.tensor.matmul(
                        qps[:],
                        wq_t[h // 2][:, dc, (h % 2) * 128 : (h % 2 + 1) * 128],
                        x_q[:, dc, :],
                        start=(dc == 0),
                        stop=(dc == NDC - 1),
                    )
                blk = slice(cq * 4, (cq + 1) * 4)
                hc = slice(h * 128, (h + 1) * 128)
                src = qps[0:64, :].rearrange("p (b q) -> p b q", b=4)
                nc.vector.tensor_mul(
                    q1[0:64, blk, hc], src, fmc.rearrange("p (b q) -> p b q", b=4)
                )
                nc.vector.tensor_mul(
                    q1[64:128, blk, hc], src, fpc.rearrange("p (b q) -> p b q", b=4)
                )
                nc.scalar.copy(
                    q2[0:64, blk, hc],
                    qps[64:128, :].rearrange("p (b q) -> p b q", b=4),
                )
                nc.scalar.copy(
                    q2[64:128, blk, hc],
                    qps[64:128, :].rearrange("p (b q) -> p b q", b=4),
                )

            # V projection -> v_sb [kpos, feat] (first consumed mid-way into
            # this quarter's attention, so it sits after Q to shorten the
            # rope -> first-scores critical chain)
            vps = ps.tile([128, 512], F32, tag="ps")
            for kb4 in range(4):
                for dc in range(NDC):
                    nc.tensor.matmul(
                        vps[:, kb4 * 128 : (kb4 + 1) * 128],
                        x_q[:, dc, kb4 * 128 : (kb4 + 1) * 128],
                        wkv_t[:, dc, 128:256],
                        start=(dc == 0),
                        stop=(dc == NDC - 1),
                    )
            nc.vector.tensor_copy(
                v_sb[:, cq * 4 : (cq + 1) * 4, 0:128],
                vps[:].rearrange("p (b q) -> p b q", b=4),
            )

            # prefetch next x chunk during this quarter's attention
            if cq + 1 < 4:
                x_tiles[cq + 1] = load_x(cq + 1)

            # finish the previous quarter's last q-block now: its transposes
            # + O-projection give the PE work while the rope tail (vector/
            # scalar) of this quarter completes
            if pending is not None:
                emit_finish(pending)
                pending = None

            # ---- attention for q-blocks of this quarter ----
            for qc in range(cq * 4, (cq + 1) * 4):
                kbs = _kblocks(qc)
                nkb = len(kbs)
                pv = [
                    pvs.tile([128, 264], F32, tag="pv", name=f"pv{qc}_{i}")
                    for i in range(2)
                ]
                # Two heads accumulate in one bank: a start=True matmul would
                # clear the co-resident head's has_written bits mid-group, so
                # zero the bank and accumulate with start=False throughout
                # (add-where-set on zeros / overwrite-where-clear both work).
                for t in pv:
                    nc.vector.memset(t[:], 0.0)

                def pv_group(kb, p_sb):
                    for h in range(HPC):
                        nc.tensor.matmul(
                            pv[h // 2][:, (h % 2) * 132 : (h % 2) * 132 + 129],
                            p_sb[:, h * 128 : (h + 1) * 128],
                            v_sb[:, kb, 0:129],
                            start=False,
                            stop=(kb == kbs[-1]),
                        )

                window = []  # pv groups trailing the score matmuls
                for mi, kb in enumerate(kbs):
                    sp = sps.tile([128, 512], F32, tag="s")
                    lo = kb * 128
                    nc.tensor.matmul(
                        sp[:], k1[:, lo : lo + 128], q1[:, qc, :],
                        start=True, stop=False,
                    )
                    nc.tensor.matmul(
                        sp[:], k2[:, lo : lo + 128], q2[:, qc, :],
                        start=False, stop=True,
                    )
                    if kb == qc - 8:
                        nc.vector.tensor_add(sp[:], sp[:], t0[:])
                    if kb == qc:
                        nc.vector.tensor_add(sp[:], sp[:], t8[:])
                    p_sb = pp.tile([128, 512], BF16, tag="p")
                    nc.scalar.activation(p_sb[:], sp[:], EXP)
                    window.append((kb, p_sb))
                    if mi == 1 and pending is not None:
                        emit_finish(pending)
                        pending = None
                    while len(window) > 2:
                        pv_group(*window.pop(0))
                if pending is not None:
                    emit_finish(pending)
                    pending = None
                for w in window:
                    pv_group(*w)
                pending = (pv, qc)

        emit_finish(pending)

    nc.compile()
    return nc


def _prep_core(inputs, c):
    x = inputs["x"]
    cos, sin = np.asarray(inputs["cos"]), np.asarray(inputs["sin"])
    mask = np.asarray(inputs["mask"])
    wq = np.asarray(inputs["wq"], dtype=np.float32)
    wk = np.asarray(inputs["wk"], dtype=np.float32)
    wv = np.asarray(inputs["wv"], dtype=np.float32)
    wo = np.asarray(inputs["wo"], dtype=np.float32)
    bf = ml_dtypes.bfloat16
    b, g = c // 4, c % 4

    # x[b] transposed -> [128p, cq, dc, 512]
    xt = np.asarray(x[b], dtype=np.float32).T  # [dim, S]
    xt = xt.reshape(NDC, 128, 4, 512).transpose(1, 2, 0, 3)
    xt = np.ascontiguousarray(xt).reshape(128, 4 * NDC * 512).astype(bf)

    # wq slice for heads 4g..4g+3 (SCALE folded). The imag half-columns are
    # additionally halved: the kernel duplicates q2/k2 across both partition
    # halves, doubling the imag contraction.
    wqs = (wq[:, g * 512 : (g + 1) * 512] * SCALE).reshape(DIM, HPC, 128).copy()
    wqs[:, :, 64:128] *= 0.5
    wqs = wqs.reshape(NDC, 128, 2, 256)
    wqs = np.ascontiguousarray(wqs.transpose(1, 2, 0, 3)).reshape(128, 2 * NDC * 256)
    # wk|wv slice for kv head g: [p, dc, 256] with cols [wk 128 | wv 128]
    wkv = np.concatenate(
        [wk[:, g * 128 : (g + 1) * 128], wv[:, g * 128 : (g + 1) * 128]], axis=1
    )
    wkv = np.ascontiguousarray(wkv.reshape(NDC, 128, 256).transpose(1, 0, 2)).reshape(
        128, NDC * 256
    )
    # wo rows for this core's heads: [p, h, 2048] tiles
    wos = wo[g * 512 : (g + 1) * 512].reshape(HPC, 128, 2048).transpose(1, 0, 2)
    wos = np.ascontiguousarray(wos).reshape(128, HPC * 2048)

    fm = np.ascontiguousarray((cos - sin).T, dtype=np.float32)
    fp_ = np.ascontiguousarray((cos + sin).T, dtype=np.float32)
    # masks transposed for the S^T layout, tiled across the 4 packed heads
    t0 = np.ascontiguousarray(
        np.tile(mask[WINDOW : WINDOW + 128, 0:128].T, (1, 4)), dtype=np.float32
    )
    t8 = np.ascontiguousarray(
        np.tile(mask[0:128, 0:128].T, (1, 4)), dtype=np.float32
    )

    return {
        "xt": xt, "wq": wqs.astype(bf), "wkv": wkv.astype(bf), "wo": wos.astype(bf),
        "fm": fm, "fp": fp_, "t0": t0, "t8": t8,
        "ident": np.eye(128, dtype=np.float32).astype(bf),
        "vones": np.ones((128, NQC), dtype=np.float32).astype(bf),
    }


def kernel(**inputs) -> np.ndarray:
    if "nc" not in _cache:
        _cache["nc"] = _build()
    nc = _cache["nc"]
    in_maps = [_prep_core(inputs, c) for c in range(8)]
    res = run_bass_kernel_spmd(nc, in_maps, core_ids=list(range(8)))
    out = np.zeros((B, S, DIM), dtype=np.float32)
    for c in range(8):
        out[c // 4] += np.asarray(res.results[c]["out"], dtype=np.float32)
    return out


# revision 44
# speedup vs baseline: 1.2641x; 1.2641x over previous
"""Distributed Bass kernel for sliding-window GQA attention on 8 TRN2 NeuronCores.

Problem: B=2, S=2048, DIM=2048, H=16, KVH=4, HD=128, WINDOW=1024 (causal
sliding window), nonstandard RoPE producing 1.5*HD score features.

Sharding (tensor-parallel on the kv-head axis, data-parallel on batch —
no collectives): core c owns (batch, kv-group) = (c//4, c%4): its 4 q-heads
and 1 kv head over the full 2048-row sequence. wq/wk/wv are column-sharded
by kv group, wo row-sharded. Each core emits a PARTIAL output projection
(its 4 heads x its wo rows); the host sums the 4 partials per batch while
unsharding — replacing the all-reduce.

Structure: scores are computed TRANSPOSED (S^T[k, q], k on partitions) with
all 4 heads packed into one N=512 matmul pair per k-block — q1/k1 are
feature-major already so this is free. The imag-half (64-dim) contraction
is duplicated across both partition halves (wq imag columns pre-halved on
the host) so both score passes run K=128 and keep fast-weight-load. The
softmax row-sums come from a ones-column appended to V (PV out [q, 129]
carries the denominator in col 128), normalization happens during the
PSUM->SBUF attn copy (per-partition scalar mul), and a single 128x128
transpose matmul per (head, q-block) feeds the O-projection.

Pipelining: attention for q-blocks 4cq..4cq+3 is interleaved right after
column-chunk cq's projections; within a q-block the PV groups trail the
score matmuls by 2 k-blocks (so the exp on the Scalar engine never stalls
the in-order PE queue), and the normalize/transpose/O-proj of q-block qc
is emitted inside q-block qc+1's loop to keep the PE fed at qc boundaries.
"""
import numpy as np
import ml_dtypes

import concourse.tile as tile
from concourse import bacc, mybir
from concourse.bass_utils import run_bass_kernel_spmd
from contextlib import ExitStack

F32 = mybir.dt.float32
BF16 = mybir.dt.bfloat16
EXP = mybir.ActivationFunctionType.Exp

B, S, DIM = 2, 2048, 2048
H, KVH, HD = 16, 4, 128
HPC = H // KVH  # heads per core (4)
WINDOW = 1024
SCALE = HD ** -0.5
NDC = DIM // 128  # 16 dim chunks
NQC = S // 128    # 16 q blocks

_cache = {}


def _kblocks(qc):
    return list(range(max(0, qc - 8), qc + 1))


def _build():
    nc = bacc.Bacc("TRN2", target_bir_lowering=False, debug=False, num_devices=8)

    xt_d = nc.dram_tensor("xt", [128, 4 * NDC * 512], BF16, kind="ExternalInput")
    wq_d = nc.dram_tensor("wq", [128, 2 * NDC * 256], BF16, kind="ExternalInput")
    wkv_d = nc.dram_tensor("wkv", [128, NDC * 256], BF16, kind="ExternalInput")
    wo_d = nc.dram_tensor("wo", [128, HPC * 2048], BF16, kind="ExternalInput")
    fmp_d = nc.dram_tensor("fmp", [128, S], BF16, kind="ExternalInput")
    t08_d = nc.dram_tensor("t08", [128, 1024], F32, kind="ExternalInput")
    idv_d = nc.dram_tensor("idv", [128, 128 + NQC], BF16, kind="ExternalInput")
    out_d = nc.dram_tensor("out", [S, DIM], BF16, kind="ExternalOutput")

    with tile.TileContext(nc) as tc, ExitStack() as ctx:
        xp = ctx.enter_context(tc.tile_pool(name="xp", bufs=3))
        wp = ctx.enter_context(tc.tile_pool(name="wp", bufs=1))
        cp = ctx.enter_context(tc.tile_pool(name="cp", bufs=1))
        qp = ctx.enter_context(tc.tile_pool(name="qp", bufs=1))
        kp = ctx.enter_context(tc.tile_pool(name="kp", bufs=1))
        vp = ctx.enter_context(tc.tile_pool(name="vp", bufs=1))
        pp = ctx.enter_context(tc.tile_pool(name="pp", bufs=6))
        ap_ = ctx.enter_context(tc.tile_pool(name="ap", bufs=8))
        atp = ctx.enter_context(tc.tile_pool(name="atp", bufs=2))
        rp = ctx.enter_context(tc.tile_pool(name="rp", bufs=8))
        op_ = ctx.enter_context(tc.tile_pool(name="op", bufs=2))
        # PSUM: 8 banks = ps(2: proj + O-proj) + sps(3: scores + attn
        # transposes) + pvs(3: PV accumulators, 2 heads per bank)
        ps = ctx.enter_context(tc.tile_pool(name="ps", bufs=2, space="PSUM"))
        sps = ctx.enter_context(tc.tile_pool(name="sps", bufs=3, space="PSUM"))
        pvs = ctx.enter_context(tc.tile_pool(name="pvs", bufs=3, space="PSUM"))

        # ---- persistent SBUF tensors ----
        q1 = qp.tile([128, NQC, 512], BF16, tag="q1")  # [feat, qblock, h*128+q]
        q2 = qp.tile([128, NQC, 512], BF16, tag="q2")  # imag, duplicated halves
        k1 = kp.tile([128, S], BF16, tag="k1")
        k2 = kp.tile([128, S], BF16, tag="k2")  # imag, duplicated halves
        v_sb = vp.tile([128, NQC, 132], BF16, tag="v")  # col 128 = ones

        # weights for phase 1 first (prologue-critical DMA order)
        wkv_t = wp.tile([128, NDC, 256], BF16, tag="wkv")  # [wk 128 | wv 128]
        nc.sync.dma_start(wkv_t[:], wkv_d[:, :])
        # prewarm the exp table-set load (~2.7us) off the first-landing DMA —
        # a later input here would head-of-line-block the Scalar FIFO
        warm = rp.tile([128, 1], F32, tag="rc", name="warm")
        nc.scalar.activation(warm[:], wkv_t[:, 0, 0:1], EXP)
        def load_x(cq):
            x_q = xp.tile([128, NDC, 512], BF16, tag="x", name=f"x{cq}")
            ndg = 8 if cq == 0 else 2
            w_dg = NDC // ndg
            for dg in range(ndg):
                nc.sync.dma_start(
                    x_q[:, dg * w_dg : (dg + 1) * w_dg, :],
                    xt_d[
                        :,
                        cq * NDC * 512 + dg * w_dg * 512 : cq * NDC * 512
                        + (dg + 1) * w_dg * 512,
                    ],
                )
            return x_q

        x_tiles = {0: load_x(0)}
        # merged constants ride just behind x0 (each dma_start costs ~600ns
        # of serial Sync issue time, so nothing sits ahead of x0-part0):
        # ident|vones, t0|t8, and the first quarter's rope factors — the
        # rope multiplies and everything behind them cascade off that last
        idv = cp.tile([128, 128 + NQC], BF16, tag="idv")
        nc.sync.dma_start(idv[:], idv_d[:, :])
        ident = idv[:, 0:128]
        von = idv[:, 128 : 128 + NQC]
        t08 = cp.tile([128, 1024], F32, tag="t08")
        nc.sync.dma_start(t08[:], t08_d[:, :])
        t0 = t08[:, 0:512]
        t8 = t08[:, 512:1024]
        fmp = cp.tile([128, S], BF16, tag="fmp")  # fm rows 0:64, fp 64:128
        nc.sync.dma_start(fmp[:, 0:512], fmp_d[:, 0:512])
        wq_t = wo_t = None
        pending = None  # (pv tiles, qc) awaiting normalize/transpose/O-proj
        window = []  # trailing (pv, kb, p_sb, is_stop) PV groups

        def pv_group(pv, kb, p_sb, is_stop):
            for h in range(HPC):
                nc.tensor.matmul(
                    pv[h // 2][:, (h % 2) * 132 : (h % 2) * 132 + 129],
                    p_sb[:, h * 128 : (h + 1) * 128],
                    v_sb[:, kb, 0:129],
                    start=False,
                    stop=is_stop,
                )

        def flush_to(pv_tiles):
            # a finish reads its PV accumulators: every trailing group that
            # targets them must be emitted first
            while window and window[0][0] is pv_tiles:
                pv_group(*window.pop(0))

        def finish_head(st, h):
            # one head's normalize + transpose; staged so each transpose
            # trails the vector recip/norm chain by a score pair
            pv = st["pv"]
            if st["at"] is None:
                st["at"] = atp.tile([128, HPC, 128], BF16, tag="at", name="at_sb")
            at_sb = st["at"]
            c0 = (h % 2) * 132
            if h % 2 == 0:
                # both heads' softmax denominators sit at cols 128/260 of the
                # shared bank: one strided reciprocal covers the pair
                rc = rp.tile([128, 2], F32, tag="rc")
                st["rc"] = rc
                nc.vector.reciprocal(rc[:], pv[h // 2][:, 128:261:132])
            rc = st["rc"]
            a_sb = ap_.tile([128, 128], BF16, tag="a")
            nc.vector.tensor_scalar_mul(
                a_sb[:], pv[h // 2][:, c0 : c0 + 128], rc[:, h % 2 : h % 2 + 1]
            )
            tps = sps.tile([128, 512], F32, tag="s", name=f"tp{h}")
            nc.tensor.matmul(tps[:, 0:128], a_sb[:], ident, start=True, stop=True)
            # alternate engines so the 4 copies gating the O-proj drain
            # two-wide instead of queuing on one engine
            if h % 2 == 0:
                nc.scalar.copy(at_sb[:, h, :], tps[:, 0:128])
            else:
                nc.vector.tensor_copy(at_sb[:, h, :], tps[:, 0:128])

        def finish_tail(st):
            at_sb = st["at"]
            qb = st["qc"] * 128
            o_sb = op_.tile([128, 2048], BF16, tag="o")
            for dn in range(4):
                ops = ps.tile([128, 512], F32, tag="ps")
                for f in range(HPC):
                    nc.tensor.matmul(
                        ops[:],
                        at_sb[:, f, :],
                        wo_t[:, f, dn * 512 : (dn + 1) * 512],
                        start=(f == 0),
                        stop=(f == HPC - 1),
                    )
                nc.vector.tensor_copy(o_sb[:, dn * 512 : (dn + 1) * 512], ops[:])
                if dn == 1:
                    nc.sync.dma_start(
                        out_d[qb : qb + 128, 0:1024], o_sb[:, 0:1024]
                    )
            # two DMAs per q-block: fused enough to keep Sync sequencing
            # cheap (~600ns per dma_start), split so the final q-block's
            # store overlaps its second O-proj half
            nc.sync.dma_start(out_d[qb : qb + 128, 1024:2048], o_sb[:, 1024:2048])

        def finish_step(st):
            # advance the staged finish by one unit; True when fully done
            h = st["step"]
            if h == 0:
                flush_to(st["pv"])
            if h < HPC:
                finish_head(st, h)
            else:
                finish_tail(st)
            st["step"] += 1
            return st["step"] > HPC

        def emit_finish(st):
            while not finish_step(st):
                pass

        for cq in range(4):
            # ---- projections for column-quarter cq ----
            x_q = x_tiles.pop(cq)
            if cq == 0:
                # wq + constants ride after the first x chunk
                wq_t = [
                    wp.tile([128, NDC, 256], BF16, tag=f"wq{i}", name=f"wq{i}")
                    for i in range(2)
                ]
                for i in range(2):
                    nc.sync.dma_start(
                        wq_t[i][:], wq_d[:, i * NDC * 256 : (i + 1) * NDC * 256]
                    )
                nc.sync.dma_start(fmp[:, 512:S], fmp_d[:, 512:S])
                nc.vector.tensor_copy(v_sb[:, :, 128:129], von.unsqueeze(2))
                wo_t = wp.tile([128, HPC, 2048], BF16, tag="wo")
                for i in range(2):
                    nc.sync.dma_start(
                        wo_t[:, i * 2 : (i + 1) * 2, :],
                        wo_d[:, i * 4096 : (i + 1) * 4096],
                    )
            cs = slice(cq * 512, (cq + 1) * 512)
            fmc, fpc = fmp[0:64, cs], fmp[64:128, cs]

            # K projection + rope
            kps = ps.tile([128, 512], F32, tag="ps")
            for dc in range(NDC):
                nc.tensor.matmul(
                    kps[:],
                    wkv_t[:, dc, 0:128],
                    x_q[:, dc, :],
                    start=(dc == 0),
                    stop=(dc == NDC - 1),
                )
            nc.vector.tensor_mul(k1[0:64, cs], kps[0:64, :], fmc)
            nc.vector.tensor_mul(k1[64:128, cs], kps[0:64, :], fpc)
            nc.scalar.copy(k2[0:64, cs], kps[64:128, :])
            nc.scalar.copy(k2[64:128, cs], kps[64:128, :])

            # Q projections + rope, packed layout [feat, qblock, h*128+q]
            for h in range(HPC):
                qps = ps.tile([128, 512], F32, tag="ps")
                for dc in range(NDC):
                    nc.tensor.matmul(
                        qps[:],
                        wq_t[h // 2][:, dc, (h % 2) * 128 : (h % 2 + 1) * 128],
                        x_q[:, dc, :],
                        start=(dc == 0),
                        stop=(dc == NDC - 1),
                    )
                blk = slice(cq * 4, (cq + 1) * 4)
                hc = slice(h * 128, (h + 1) * 128)
                src = qps[0:64, :].rearrange("p (b q) -> p b q", b=4)
                nc.vector.tensor_mul(
                    q1[0:64, blk, hc], src, fmc.rearrange("p (b q) -> p b q", b=4)
                )
                nc.vector.tensor_mul(
                    q1[64:128, blk, hc], src, fpc.rearrange("p (b q) -> p b q", b=4)
                )
                nc.scalar.copy(
                    q2[0:64, blk, hc],
                    qps[64:128, :].rearrange("p (b q) -> p b q", b=4),
                )
                nc.scalar.copy(
                    q2[64:128, blk, hc],
                    qps[64:128, :].rearrange("p (b q) -> p b q", b=4),
                )

            # V projection -> v_sb [kpos, feat] (first consumed mid-way into
            # this quarter's attention, so it sits after Q to shorten the
            # rope -> first-scores critical chain)
            vps = ps.tile([128, 512], F32, tag="ps")
            for kb4 in range(4):
                for dc in range(NDC):
                    nc.tensor.matmul(
                        vps[:, kb4 * 128 : (kb4 + 1) * 128],
                        x_q[:, dc, kb4 * 128 : (kb4 + 1) * 128],
                        wkv_t[:, dc, 128:256],
                        start=(dc == 0),
                        stop=(dc == NDC - 1),
                    )
            nc.vector.tensor_copy(
                v_sb[:, cq * 4 : (cq + 1) * 4, 0:128],
                vps[:].rearrange("p (b q) -> p b q", b=4),
            )

            # prefetch next x chunk during this quarter's attention
            if cq + 1 < 4:
                x_tiles[cq + 1] = load_x(cq + 1)

            # finish the previous quarter's last q-block now: its transposes
            # + O-projection give the PE work while the rope tail (vector/
            # scalar) of this quarter completes
            if pending is not None:
                emit_finish(pending)
                pending = None

            # ---- attention for q-blocks of this quarter ----
            for qc in range(cq * 4, (cq + 1) * 4):
                kbs = _kblocks(qc)
                nkb = len(kbs)
                pv = [
                    pvs.tile([128, 264], F32, tag="pv", name=f"pv{qc}_{i}")
                    for i in range(2)
                ]
                # Two heads accumulate in one bank: a start=True matmul would
                # clear the co-resident head's has_written bits mid-group, so
                # zero the bank and accumulate with start=False throughout
                # (add-where-set on zeros / overwrite-where-clear both work).
                for t in pv:
                    nc.vector.memset(t[:], 0.0)

                for mi, kb in enumerate(kbs):
                    sp = sps.tile([128, 512], F32, tag="s")
                    lo = kb * 128
                    nc.tensor.matmul(
                        sp[:], k1[:, lo : lo + 128], q1[:, qc, :],
                        start=True, stop=False,
                    )
                    nc.tensor.matmul(
                        sp[:], k2[:, lo : lo + 128], q2[:, qc, :],
                        start=False, stop=True,
                    )
                    if kb == qc - 8:
                        nc.vector.tensor_add(sp[:], sp[:], t0)
                    if kb == qc:
                        nc.vector.tensor_add(sp[:], sp[:], t8)
                    p_sb = pp.tile([128, 512], BF16, tag="p")
                    nc.scalar.activation(p_sb[:], sp[:], EXP)
                    # the PV group trails by 2 k-blocks, carried across
                    # q-block boundaries so score matmuls always cover the
                    # exp latency
                    window.append((pv, kb, p_sb, kb == kbs[-1]))
                    # the previous q-block's finish advances one stage per
                    # k-block: each transpose trails the vector recip/norm
                    # chain with a score pair of PE cover in between
                    if pending is not None and mi >= 1:
                        if finish_step(pending):
                            pending = None
                    while len(window) > 3:
                        pv_group(*window.pop(0))
                while pending is not None:
                    if finish_step(pending):
                        pending = None
                pending = {"pv": pv, "qc": qc, "at": None, "rc": None, "step": 0}

        emit_finish(pending)

    nc.compile()
    return nc


def _prep_core(inputs, c):
    x = inputs["x"]
    cos, sin = np.asarray(inputs["cos"]), np.asarray(inputs["sin"])
    mask = np.asarray(inputs["mask"])
    wq = np.asarray(inputs["wq"], dtype=np.float32)
    wk = np.asarray(inputs["wk"], dtype=np.float32)
    wv = np.asarray(inputs["wv"], dtype=np.float32)
    wo = np.asarray(inputs["wo"], dtype=np.float32)
    bf = ml_dtypes.bfloat16
    b, g = c // 4, c % 4

    # x[b] transposed -> [128p, cq, dc, 512]
    xt = np.asarray(x[b], dtype=np.float32).T  # [dim, S]
    xt = xt.reshape(NDC, 128, 4, 512).transpose(1, 2, 0, 3)
    xt = np.ascontiguousarray(xt).reshape(128, 4 * NDC * 512).astype(bf)

    # wq slice for heads 4g..4g+3 (SCALE folded). The imag half-columns are
    # additionally halved: the kernel duplicates q2/k2 across both partition
    # halves, doubling the imag contraction.
    wqs = (wq[:, g * 512 : (g + 1) * 512] * SCALE).reshape(DIM, HPC, 128).copy()
    wqs[:, :, 64:128] *= 0.5
    wqs = wqs.reshape(NDC, 128, 2, 256)
    wqs = np.ascontiguousarray(wqs.transpose(1, 2, 0, 3)).reshape(128, 2 * NDC * 256)
    # wk|wv slice for kv head g: [p, dc, 256] with cols [wk 128 | wv 128]
    wkv = np.concatenate(
        [wk[:, g * 128 : (g + 1) * 128], wv[:, g * 128 : (g + 1) * 128]], axis=1
    )
    wkv = np.ascontiguousarray(wkv.reshape(NDC, 128, 256).transpose(1, 0, 2)).reshape(
        128, NDC * 256
    )
    # wo rows for this core's heads: [p, h, 2048] tiles
    wos = wo[g * 512 : (g + 1) * 512].reshape(HPC, 128, 2048).transpose(1, 0, 2)
    wos = np.ascontiguousarray(wos).reshape(128, HPC * 2048)

    # fm rows 0:64, fp rows 64:128 (one tensor -> one DMA)
    fmp = np.concatenate([(cos - sin).T, (cos + sin).T], axis=0)
    fmp = np.ascontiguousarray(fmp, dtype=np.float32).astype(bf)
    # masks transposed for the S^T layout, tiled across the 4 packed heads
    t08 = np.concatenate(
        [
            np.tile(mask[WINDOW : WINDOW + 128, 0:128].T, (1, 4)),
            np.tile(mask[0:128, 0:128].T, (1, 4)),
        ],
        axis=1,
    )
    t08 = np.ascontiguousarray(t08, dtype=np.float32)
    idv = np.concatenate(
        [np.eye(128, dtype=np.float32), np.ones((128, NQC), dtype=np.float32)],
        axis=1,
    ).astype(bf)

    return {
        "xt": xt, "wq": wqs.astype(bf), "wkv": wkv.astype(bf), "wo": wos.astype(bf),
        "fmp": fmp, "t08": t08, "idv": np.ascontiguousarray(idv),
    }


def kernel(**inputs) -> np.ndarray:
    if "nc" not in _cache:
        _cache["nc"] = _build()
    nc = _cache["nc"]
    in_maps = [_prep_core(inputs, c) for c in range(8)]
    res = run_bass_kernel_spmd(nc, in_maps, core_ids=list(range(8)))
    out = np.zeros((B, S, DIM), dtype=np.float32)
    for c in range(8):
        out[c // 4] += np.asarray(res.results[c]["out"], dtype=np.float32)
    return out
